# revision 4
# baseline (speedup 1.0000x reference)
"""Gaussian voxel renderer on 8 trn2 NeuronCores.

Math: for voxel p and gaussian n (in input order),
    alpha[p,n] = opa_n * exp(-0.5 * (c_p - mu_n)^T A_n (c_p - mu_n)),  A = inv cov
    w[p,n] = alpha[p,n] * prod_{j<n} (1 - alpha[p,j])
    out[p,:] = sum_n w[p,n] * feat[n,:]

Device pipeline (voxels on partitions, gaussians on the free axis), with the
compositing telescoped to  out = f0 + sum_n S_n * g_n,  S = inclusive
cumprod(1-alpha), g = diff(features):
    u = basis^T @ G            PE, 3-term fp16 split (fp32-grade accuracy)
    alpha = exp(u)             ACT
    m = 1 - alpha              GPSIMD tensor_scalar
    S = cumprod(m)             DVE tensor_tensor_scan, fp32 state, fp16 out
    S^T                        PE fp16 transposes -> PSUM, DVE copy -> SBUF
    r = S^T.T @ g              PE fp16, accumulated over 4 gaussian chunks
Voxel slabs are sharded across the 8 cores; per-gaussian parameters are
replicated. Host does the tiny per-gaussian precompute (quat->rot, 3x3
inverse, fp16 hi/lo splits) in float64 and the final gather/deinterleave.
"""
import numpy as np

import concourse.bacc as bacc
import concourse.tile as tile
import concourse.mybir as mybir
from concourse.bass_utils import run_bass_kernel_spmd
from concourse.masks import make_identity

F32 = mybir.dt.float32
F16 = mybir.dt.float16
AF = mybir.ActivationFunctionType
ALU = mybir.AluOpType

H, W, D = 96, 96, 16
N, F = 512, 32
NCORES = 8
P_TOTAL = H * W * D
P_LOCAL = P_TOTAL // NCORES          # 18432
TILES = P_LOCAL // 128               # 144
NCHUNK = N // 128                    # 4
LO_SCALE = 4096.0                    # 2**12, fp16 low-part scaling

GROUP = 4                            # tiles per r_ps bank / out-copy batch
OUT_CHUNKS = 4                       # output DMA granularity


def _build_nc():
    nc = bacc.Bacc("TRN2", target_bir_lowering=False, debug=False)
    bhi_d = nc.dram_tensor("basis_hi", [10, P_LOCAL], F16, kind="ExternalInput")
    blo_d = nc.dram_tensor("basis_lo", [10, P_LOCAL], F16, kind="ExternalInput")
    ghi_d = nc.dram_tensor("G_hi", [10, N], F16, kind="ExternalInput")
    glo_d = nc.dram_tensor("G_lo", [10, N], F16, kind="ExternalInput")
    ghis_d = nc.dram_tensor("G_his", [10, N], F16, kind="ExternalInput")
    gfh_d = nc.dram_tensor("gfeat_hi", [128, NCHUNK * F], F16, kind="ExternalInput")
    gfl_d = nc.dram_tensor("gfeat_lo", [128, NCHUNK * F], F16, kind="ExternalInput")
    rend_d = nc.dram_tensor("rend", [128, TILES * F], F32, kind="ExternalOutput")

    tpq = TILES // OUT_CHUNKS
    with tile.TileContext(nc) as tc:
        with tc.tile_pool(name="const", bufs=1) as const, \
             tc.tile_pool(name="work", bufs=3) as work, \
             tc.tile_pool(name="outp", bufs=2) as outp, \
             tc.tile_pool(name="ps_u", bufs=2, space="PSUM") as ps_u, \
             tc.tile_pool(name="ps_t", bufs=2, space="PSUM") as ps_t, \
             tc.tile_pool(name="ps_r", bufs=2, space="PSUM") as ps_r:

            bhi_sb = const.tile([10, P_LOCAL], F16)
            nc.sync.dma_start(bhi_sb[:], bhi_d[:])
            blo_sb = const.tile([10, P_LOCAL], F16)
            nc.sync.dma_start(blo_sb[:], blo_d[:])
            ghi_sb = const.tile([10, N], F16)
            nc.sync.dma_start(ghi_sb[:], ghi_d[:])
            glo_sb = const.tile([10, N], F16)
            nc.sync.dma_start(glo_sb[:], glo_d[:])
            ghis_sb = const.tile([10, N], F16)
            nc.sync.dma_start(ghis_sb[:], ghis_d[:])
            gfh_sb = const.tile([128, NCHUNK * F], F16)
            nc.sync.dma_start(gfh_sb[:], gfh_d[:])
            gfl_sb = const.tile([128, NCHUNK * F], F16)
            nc.sync.dma_start(gfl_sb[:], gfl_d[:])
            ident = const.tile([128, 128], F16)
            make_identity(nc, ident[:])

            for q in range(OUT_CHUNKS):
                out_sb = outp.tile([128, tpq * F], F32, tag="out")
                for g in range(tpq // GROUP):
                    r_ps = ps_r.tile([128, GROUP * F], F32, tag="r")
                    r2_ps = ps_r.tile([128, GROUP * F], F32, tag="r2")
                    for j in range(GROUP):
                        it = g * GROUP + j
                        i = q * tpq + it
                        sl = slice(i * 128, (i + 1) * 128)
                        u_ps = ps_u.tile([128, N], F32, tag="u")
                        nc.tensor.matmul(u_ps[:], bhi_sb[:, sl], ghi_sb[:],
                                         start=True, stop=False)
                        nc.tensor.matmul(u_ps[:], bhi_sb[:, sl], glo_sb[:],
                                         start=False, stop=False)
                        nc.tensor.matmul(u_ps[:], blo_sb[:, sl], ghis_sb[:],
                                         start=False, stop=True)
                        alpha = work.tile([128, N], F32, tag="alpha")
                        nc.scalar.activation(alpha[:], u_ps[:], AF.Exp)
                        m = work.tile([128, N], F32, tag="m")
                        nc.gpsimd.tensor_scalar(m[:], alpha[:], -1.0, 1.0,
                                                op0=ALU.mult, op1=ALU.add)
                        S = work.tile([128, N], F16, tag="S")
                        nc.vector.tensor_tensor_scan(S[:], m[:], m[:], 1.0,
                                                     op0=ALU.mult,
                                                     op1=ALU.bypass)
                        st_ps = ps_t.tile([128, N], F16, tag="st")
                        for c in range(NCHUNK):
                            nc.tensor.transpose(
                                st_ps[:, c * 128:(c + 1) * 128],
                                S[:, c * 128:(c + 1) * 128], ident[:])
                        ST = work.tile([128, N], F16, tag="ST")
                        nc.vector.tensor_copy(ST[:], st_ps[:])
                        for c in range(NCHUNK):
                            nc.tensor.matmul(r_ps[:, j * F:(j + 1) * F],
                                             ST[:, c * 128:(c + 1) * 128],
                                             gfh_sb[:, c * F:(c + 1) * F],
                                             start=(c == 0),
                                             stop=(c == NCHUNK - 1))
                            nc.tensor.matmul(r2_ps[:, j * F:(j + 1) * F],
                                             ST[:, c * 128:(c + 1) * 128],
                                             gfl_sb[:, c * F:(c + 1) * F],
                                             start=(c == 0),
                                             stop=(c == NCHUNK - 1))
                    osl = out_sb[:, g * GROUP * F:(g + 1) * GROUP * F]
                    nc.scalar.activation(osl, r_ps[:], AF.Copy)
                    nc.vector.scalar_tensor_tensor(
                        osl, r2_ps[:], 1.0 / LO_SCALE, osl,
                        op0=ALU.mult, op1=ALU.add)
                nc.sync.dma_start(rend_d[:, q * tpq * F:(q + 1) * tpq * F],
                                  out_sb[:])
    nc.compile()
    return nc


_NC_CACHE = None


def _get_nc():
    global _NC_CACHE
    if _NC_CACHE is None:
        _NC_CACHE = _build_nc()
    return _NC_CACHE


def _host_prep(means, scales, rotations, opacities, features, camera_transform,
               coord_grid):
    f8 = np.float64
    means = means.astype(f8)
    scales = scales.astype(f8)
    q = rotations.astype(f8)
    opa = opacities.astype(f8)[:, 0]
    T = camera_transform.astype(f8)

    homo = np.concatenate([means, np.ones((N, 1))], axis=1) @ T.T
    mu = homo[:, :3] / homo[:, 3:4]

    q = q / np.linalg.norm(q, axis=1, keepdims=True)
    w, x, y, z = q[:, 0], q[:, 1], q[:, 2], q[:, 3]
    R = np.stack([
        np.stack([1 - 2 * (y * y + z * z), 2 * (x * y - w * z), 2 * (x * z + w * y)], 1),
        np.stack([2 * (x * y + w * z), 1 - 2 * (x * x + z * z), 2 * (y * z - w * x)], 1),
        np.stack([2 * (x * z - w * y), 2 * (y * z + w * x), 1 - 2 * (x * x + y * y)], 1),
    ], axis=1)
    RS = R * scales[:, None, :]
    cov = np.einsum('nik,njk->nij', RS, RS)
    A = np.linalg.inv(cov)

    Am = np.einsum('nij,nj->ni', A, mu)
    const = -0.5 * np.einsum('ni,ni->n', mu, Am) + np.log(np.maximum(opa, 1e-300))
    G = np.empty((10, N), f8)
    G[0] = -0.5 * A[:, 0, 0]
    G[1] = -0.5 * A[:, 1, 1]
    G[2] = -0.5 * A[:, 2, 2]
    G[3] = -A[:, 0, 1]
    G[4] = -A[:, 0, 2]
    G[5] = -A[:, 1, 2]
    G[6] = Am[:, 0]
    G[7] = Am[:, 1]
    G[8] = Am[:, 2]
    G[9] = np.maximum(const, -60000.0)   # keep within fp16 range

    coords = coord_grid.astype(f8).reshape(-1, 3)
    cx, cy, cz = coords[:, 0], coords[:, 1], coords[:, 2]
    basis = np.stack([cx * cx, cy * cy, cz * cz, cx * cy, cx * cz, cy * cz,
                      cx, cy, cz, np.ones_like(cx)], axis=0)  # [10, P]

    h16 = np.float16
    b_hi = basis.astype(h16)
    b_lo = ((basis - b_hi.astype(f8)) * LO_SCALE).astype(h16)
    G_hi = G.astype(h16)
    G_lo = (G - G_hi.astype(f8)).astype(h16)
    G_his = (G_hi.astype(f8) / LO_SCALE).astype(h16)

    feats = features.astype(f8)
    g = np.empty_like(feats)
    g[:-1] = feats[1:] - feats[:-1]
    g[-1] = -feats[-1]
    g_dev = g.reshape(NCHUNK, 128, F).transpose(1, 0, 2).reshape(128, NCHUNK * F)
    gf_hi = np.ascontiguousarray(g_dev).astype(h16)
    gf_lo = np.ascontiguousarray(
        (g_dev - gf_hi.astype(f8)) * LO_SCALE).astype(h16)
    f0 = feats[0]

    return b_hi, b_lo, G_hi, G_lo, G_his, gf_hi, gf_lo, f0.astype(np.float32)


def kernel(means, scales, rotations, opacities, features, camera_transform,
           coord_grid):
    b_hi, b_lo, G_hi, G_lo, G_his, gf_hi, gf_lo, f0 = _host_prep(
        means, scales, rotations, opacities, features, camera_transform,
        coord_grid)
    nc = _get_nc()
    in_maps = []
    for c in range(NCORES):
        sl = slice(c * P_LOCAL, (c + 1) * P_LOCAL)
        in_maps.append({
            "basis_hi": np.ascontiguousarray(b_hi[:, sl]),
            "basis_lo": np.ascontiguousarray(b_lo[:, sl]),
            "G_hi": G_hi, "G_lo": G_lo, "G_his": G_his,
            "gfeat_hi": gf_hi, "gfeat_lo": gf_lo,
        })
    res = run_bass_kernel_spmd(nc, in_maps, core_ids=list(range(NCORES)))
    parts = []
    for c in range(NCORES):
        r = res.results[c]["rend"]                      # [128, TILES*F]
        part = r.reshape(128, TILES, F).transpose(1, 0, 2).reshape(P_LOCAL, F)
        parts.append(part)
    out = np.concatenate(parts, axis=0) + f0[None, :]
    return out.reshape(H, W, D, F).astype(np.float32)
